# revision 7
# baseline (speedup 1.0000x reference)
"""Trainium2 Bass kernel for nn_DynamicLSTM (scatter_memory).

Self-contained: kernel(**inputs) -> (opt, agent_action, agent_prob).

Strategy: data-parallel over batch (B=1024) across 8 NeuronCores, 128 rows
per core (= SBUF partitions). The T=255 recurrent scan runs fully unrolled
on-device. Per step:
  - policy logits = x_t @ Wx'.T + h @ Wh'.T  (Wx' = fc2@fc1_x, Wh' = fc2@fc1_h,
    folded on host; the two-layer policy MLP has no nonlinearity between)
  - + host-precomputed Gumbel noise (exactly jax.random.categorical's),
    argmax -> action
  - rolling 16-slot ring of (c, h) states in SBUF; per-row gather by action
    implemented as a 16-slot mask-multiply-accumulate on the vector engine
  - LSTM cell with gates from fp32 PE matmuls (x2h moving-weights with x^T
    stationary; h2h with gathered-h^T stationary via PE transpose)
  - log-prob computation deferred and batched per chunk.
"""
import functools
import time

import numpy as np

import concourse.bass as bass
import concourse.mybir as mybir
import concourse.tile as tile
from concourse.vector_clock import ScopedClock, VectorClock

F32 = mybir.dt.float32
I32 = mybir.dt.int32
U32 = mybir.dt.uint32

B, T, D, H, U, A, O = 1024, 256, 128, 256, 64, 16, 128
NCORES = 8
BL = B // NCORES          # 128 rows per core
NSTEP = T - 1             # 255 scan steps
CH = 64                   # chunk size for x/gumbel streaming + logp batching
AF = mybir.ActivationFunctionType
AL = mybir.AluOpType

# weight pack layout (f32 elements per partition)
W_X2H = 0                 # [128, 1024] x2h moving (d-part)
W_H2H = 1024              # [128, 2*1024] h2h moving (h%128-part, k-tile major)
W_XP = W_H2H + 2048       # [128, 16] policy x weights
W_HP = W_XP + 16          # [128, 2*16] policy h weights (k-tile major)
W_OUT = W_HP + 32         # [128, 2*128] out_w moving (k-tile major)
W_IOTA = W_OUT + 256      # [128, 32] iota 0..31
W_END = W_IOTA + 32


def _split_sync_waits(nc, max_waits=1):
    """Walrus (this build) rejects instructions carrying more than one
    sync-wait command; move excess waits onto same-engine nops."""
    ctr = 0
    for f in nc.m.functions:
        for blk in f.blocks:
            out = []
            for inst in blk.instructions:
                si = inst.sync_info
                if si is not None and len(si.on_wait) > max_waits:
                    waits = list(si.on_wait)
                    head, tail = waits[:-max_waits], waits[-max_waits:]
                    for w in head:
                        nop = mybir.InstNoOp(name=f"waitsplit_{ctr}", ins=[], outs=[])
                        ctr += 1
                        nop.engine = inst.engine
                        nop.sync_info = mybir.SyncInfo(on_wait=[w], on_update=[])
                        out.append(nop)
                    inst.sync_info = mybir.SyncInfo(
                        on_wait=tail, on_update=list(si.on_update))
                out.append(inst)
            blk.instructions[:] = out
    return ctr


class _ChunkedDrainTC(tile.TileContext):
    """Chunk kernel-tail drain waits over many sync nops (walrus limits the
    number of sync-wait commands a single instruction may carry)."""

    def _drain_and_barrier(self, tick_clock, wait_clock):
        import re
        ticks = [int(x) for x in re.findall(r"-?\d+", repr(tick_clock.global_clock))]
        for p, t in enumerate(ticks):
            if t > 0:
                sub = [0] * len(ticks)
                sub[p] = t
                nop = self.nc.sync.nop()
                wait_clock.add_sem_waits(nop.ins, ScopedClock({None: VectorClock(sub)}))
        self.nc.sync.drain()
        self.nc.all_engine_barrier()
        assert self.sems is not None
        popped = self.nc._tile_sem_poison_stack.pop()
        assert popped is self._sem_poison
        self.nc.clear_and_free_semaphores(list(self.sems.allocated().values()))
        self.nc.all_engine_barrier()


def build_bass(nsteps=NSTEP):
    nc = bass.Bass("TRN2", target_bir_lowering=False, debug=False)
    nx = nsteps + 1  # number of x timesteps used (prologue + nsteps)

    XT = nc.dram_tensor("XT", [nx, D, BL], F32, kind="ExternalInput").ap()
    GUM = nc.dram_tensor("GUM", [BL, nsteps * A], F32, kind="ExternalInput").ap()
    WPK = nc.dram_tensor("WPK", [128, W_END], F32, kind="ExternalInput").ap()
    OUTF = nc.dram_tensor("OUTF", [BL, O + nsteps], F32, kind="ExternalOutput").ap()
    OUTI = nc.dram_tensor("OUTI", [BL, nsteps], I32, kind="ExternalOutput").ap()

    def sb(name, shape, dt=F32):
        return nc.alloc_sbuf_tensor(name, shape, dt).ap()

    def ps(name, shape):
        return nc.alloc_psum_tensor(name, shape, F32).ap()

    with _ChunkedDrainTC(nc) as tc:
        W = sb("W", [128, W_END])
        # x chunks (double buffered), t-slice layout [d-part, b]
        nchunk = (nx + CH - 1) // CH
        xt_buf = [sb(f"xt{i}", [128, CH * BL]) for i in range(2)]
        gm_buf = [sb(f"gm{i}", [128, CH * A]) for i in range(2)]
        RING = sb("RING", [128, 16 * 2 * H])      # per slot: c(256) | h(256)
        S = sb("S", [128, 1536])                  # sig(768) | tg(256) | wc(256) | wh(256)
        P = sb("P", [128, 512])
        A1 = sb("A1", [128, 512])
        whT = sb("whT", [128, 256])
        hT = sb("hT", [128, 256])
        TC = sb("TCt", [128, H])
        LG = sb("LG", [128, A])
        mx = sb("mx", [128, 8])
        mi = sb("mi", [128, 8], U32)
        m32 = sb("m32", [128, 32])
        m16 = sb("m16", [128, 16])
        lstore = sb("lstore", [128, nsteps * A])
        act_f = sb("act_f", [128, nsteps])
        act_i = sb("act_i", [128, nsteps], I32)
        probs = sb("probs", [128, nsteps])
        # logp batch scratch
        sc0 = sb("sc0", [128, CH * A])
        sc1 = sb("sc1", [128, CH * A])
        r0 = sb("r0", [128, CH])
        r1 = sb("r1", [128, CH])
        r2 = sb("r2", [128, CH])
        opt_t = sb("opt_t", [128, O])
        eps = sb("eps", [128, 1])

        g_ps = ps("g_ps", [128, 1024])
        l_ps = ps("l_ps", [128, 16])
        t_ps = ps("t_ps", [128, 256])
        o_ps = ps("o_ps", [128, 128])

        ident = sb("ident", [128, 128])
        from concourse.masks import make_identity
        make_identity(nc, ident)

        nc.sync.dma_start(W, WPK)
        iota32 = W[:, W_IOTA:W_IOTA + 32]

        def load_chunk(c):
            i0 = c * CH
            ilen = min(CH, nx - i0)
            nc.sync.dma_start(
                xt_buf[c % 2][:, 0:ilen * BL].rearrange("p (t b) -> p t b", t=ilen),
                XT[i0:i0 + ilen].rearrange("t d b -> d t b"))
            j0 = c * CH
            jlen = min(CH, nsteps - j0)
            if jlen > 0:
                nc.sync.dma_start(gm_buf[c % 2][:, 0:jlen * A], GUM[:, j0 * A:(j0 + jlen) * A])

        load_chunk(0)
        nc.vector.memset(RING[:], 0.0)

        def xt_at(i):
            return xt_buf[(i // CH) % 2][:, (i % CH) * BL:(i % CH + 1) * BL]

        def gum_at(j):
            return gm_buf[(j // CH) % 2][:, (j % CH) * A:(j % CH + 1) * A]

        def transpose_256(src_ap, dst_sbuf):
            # src [128, 256] -> dst [128, 2*128] (k-tile major transpose)
            nc.tensor.transpose(t_ps[:, 0:128], src_ap[:, 0:128], ident[:])
            nc.tensor.transpose(t_ps[:, 128:256], src_ap[:, 128:256], ident[:])
            nc.vector.tensor_copy(dst_sbuf[:], t_ps[:])

        def cell(j, with_h2h):
            """gates for step j are in g_ps; produce s_{j+1} in RING."""
            snew = (j + 1) % 16
            nc.scalar.activation(S[:, 0:768], g_ps[:, 0:768], AF.Sigmoid)
            nc.scalar.activation(S[:, 768:1024], g_ps[:, 768:1024], AF.Tanh)
            if with_h2h:
                # P = [sig_i, sig_f] * [tg, wc]
                nc.vector.tensor_tensor(P[:], S[:, 0:512], S[:, 768:1280], AL.mult)
                nc.vector.tensor_tensor(
                    RING[:, snew * 512:snew * 512 + 256], P[:, 0:256], P[:, 256:512], AL.add)
            else:
                # prologue: c = sig_i * tg
                nc.vector.tensor_tensor(
                    RING[:, snew * 512:snew * 512 + 256], S[:, 0:256], S[:, 768:1024], AL.mult)
            nc.scalar.activation(TC[:], RING[:, snew * 512:snew * 512 + 256], AF.Tanh)
            nc.vector.tensor_tensor(
                RING[:, snew * 512 + 256:snew * 512 + 512], S[:, 512:768], TC[:], AL.mult)
            # hT for the next step's policy (and the final output head)
            transpose_256(RING[:, snew * 512 + 256:snew * 512 + 512], hT)

        # ---- prologue: s_0 = cell(x_0, 0, 0) -> RING slot 0
        x0 = xt_at(0)
        nc.tensor.matmul(g_ps[:, 0:512], lhsT=x0, rhs=W[:, W_X2H:W_X2H + 512],
                         start=True, stop=True)
        nc.tensor.matmul(g_ps[:, 512:1024], lhsT=x0, rhs=W[:, W_X2H + 512:W_X2H + 1024],
                         start=True, stop=True)
        cell(-1, with_h2h=False)

        # ---- scan
        for j in range(nsteps):
            xi = j + 1
            if xi % CH == 0 and xi // CH < nchunk:
                load_chunk(xi // CH)
            if xi % CH == 1 and xi // CH + 1 < nchunk:
                load_chunk(xi // CH + 1)
            xt = xt_at(xi)
            # policy logits: x part + h part (hT holds h_j)
            nc.tensor.matmul(l_ps[:], lhsT=xt, rhs=W[:, W_XP:W_XP + 16],
                             start=True, stop=False)
            nc.tensor.matmul(l_ps[:], lhsT=hT[:, 0:128],
                             rhs=W[:, W_HP:W_HP + 16], start=False, stop=False)
            nc.tensor.matmul(l_ps[:], lhsT=hT[:, 128:256],
                             rhs=W[:, W_HP + 16:W_HP + 32], start=False, stop=True)
            # gates: x2h early (accumulated in g_ps banks)
            nc.tensor.matmul(g_ps[:, 0:512], lhsT=xt, rhs=W[:, W_X2H:W_X2H + 512],
                             start=True, stop=False)
            nc.tensor.matmul(g_ps[:, 512:1024], lhsT=xt,
                             rhs=W[:, W_X2H + 512:W_X2H + 1024], start=True, stop=False)
            # sampling
            nc.vector.scalar_tensor_tensor(LG[:], l_ps[:], 0.0, gum_at(j),
                                           AL.add, AL.add)
            nc.vector.tensor_copy(lstore[:, j * A:(j + 1) * A], l_ps[:])
            nc.vector.max(mx[:], LG[:])
            nc.vector.max_index(mi[:], mx[:], LG[:])
            nc.vector.tensor_copy(act_f[:, j:j + 1], mi[:, 0:1])
            nc.vector.tensor_copy(act_i[:, j:j + 1], mi[:, 0:1])
            # ring-slot one-hot over 32 then fold mod 16
            cj = float((j + 1) % 16)
            nc.vector.tensor_scalar(m32[:], iota32, act_f[:, j:j + 1], cj,
                                    AL.subtract, AL.is_equal)
            nc.vector.tensor_tensor(m16[:], m32[:, 0:16], m32[:, 16:32], AL.add)
            # gather: 16-slot mask-multiply-accumulate -> S[:, 1024:1536]
            nc.vector.tensor_scalar(A1[:], RING[:, 0:512], m16[:, 0:1], None, AL.mult)
            for r in range(1, 16):
                dst = S[:, 1024:1536] if r % 2 == 1 else A1[:]
                src = A1[:] if r % 2 == 1 else S[:, 1024:1536]
                nc.vector.scalar_tensor_tensor(
                    dst, RING[:, r * 512:(r + 1) * 512], m16[:, r:r + 1], src,
                    AL.mult, AL.add)
            # wc||wh landed in S[:, 1024:1536] (r=15 odd)
            transpose_256(S[:, 1280:1536], whT)
            # h2h accumulate into gates
            nc.tensor.matmul(g_ps[:, 0:512], lhsT=whT[:, 0:128],
                             rhs=W[:, W_H2H:W_H2H + 512], start=False, stop=False)
            nc.tensor.matmul(g_ps[:, 512:1024], lhsT=whT[:, 0:128],
                             rhs=W[:, W_H2H + 512:W_H2H + 1024], start=False, stop=False)
            nc.tensor.matmul(g_ps[:, 0:512], lhsT=whT[:, 128:256],
                             rhs=W[:, W_H2H + 1024:W_H2H + 1536], start=False, stop=True)
            nc.tensor.matmul(g_ps[:, 512:1024], lhsT=whT[:, 128:256],
                             rhs=W[:, W_H2H + 1536:W_H2H + 2048], start=False, stop=True)
            cell(j, with_h2h=True)

            # ---- deferred logp for a finished chunk
            if (j + 1) % CH == 0 or j == nsteps - 1:
                c0 = (j // CH) * CH
                n = j + 1 - c0
                L = lstore[:, c0 * A:(j + 1) * A]
                L3 = L.rearrange("p (t a) -> p t a", a=A)
                nc.vector.tensor_reduce(r0[:, 0:n], L3, mybir.AxisListType.X, AL.max)
                nc.vector.tensor_tensor(
                    sc0[:, 0:n * A].rearrange("p (t a) -> p t a", a=A), L3,
                    r0[:, 0:n].unsqueeze(2).broadcast_to([128, n, A]), AL.subtract)
                nc.scalar.activation(sc1[:, 0:n * A], sc0[:, 0:n * A], AF.Exp)
                nc.vector.tensor_reduce(
                    r1[:, 0:n], sc1[:, 0:n * A].rearrange("p (t a) -> p t a", a=A),
                    mybir.AxisListType.X, AL.add)
                nc.scalar.activation(r1[:, 0:n], r1[:, 0:n], AF.Ln)
                # mask of chosen actions; sc0 still holds (L - rmax)
                nc.vector.tensor_tensor(
                    sc1[:, 0:n * A].rearrange("p (t a) -> p t a", a=A),
                    W[:, W_IOTA:W_IOTA + 16].unsqueeze(1).broadcast_to([128, n, A]),
                    act_f[:, c0:j + 1].unsqueeze(2).broadcast_to([128, n, A]),
                    AL.is_equal)
                nc.vector.tensor_tensor(sc0[:, 0:n * A], sc0[:, 0:n * A],
                                        sc1[:, 0:n * A], AL.mult)
                nc.vector.tensor_reduce(
                    r2[:, 0:n], sc0[:, 0:n * A].rearrange("p (t a) -> p t a", a=A),
                    mybir.AxisListType.X, AL.add)
                # logp = (sel - rmax...) wait: sel already = logit[a] - rmax
                nc.vector.tensor_tensor(probs[:, c0:j + 1], r2[:, 0:n], r1[:, 0:n],
                                        AL.subtract)

        # ---- epilogue: opt = softmax(h_last @ out_w.T); hT holds h_last^T
        nc.tensor.matmul(o_ps[:], lhsT=hT[:, 0:128], rhs=W[:, W_OUT:W_OUT + 128],
                         start=True, stop=False)
        nc.tensor.matmul(o_ps[:], lhsT=hT[:, 128:256], rhs=W[:, W_OUT + 128:W_OUT + 256],
                         start=False, stop=True)
        nc.vector.tensor_reduce(eps[:], o_ps[:], mybir.AxisListType.X, AL.max)
        nc.vector.tensor_scalar(eps[:], eps[:], -1.0, None, AL.mult)
        nc.scalar.activation(opt_t[:], o_ps[:], AF.Exp, bias=eps[:])
        nc.vector.tensor_reduce(eps[:], opt_t[:], mybir.AxisListType.X, AL.add)
        nc.vector.reciprocal(eps[:], eps[:])
        nc.vector.tensor_scalar(opt_t[:], opt_t[:], eps[:], None, AL.mult)

        nc.sync.dma_start(OUTF[:, 0:O], opt_t[:])
        nc.sync.dma_start(OUTF[:, O:O + nsteps], probs[:])
        nc.sync.dma_start(OUTI, act_i[:])
    return nc


# ---------------------------------------------------------------- host side

def _prep_weights(fc1_w, fc2_w, x2h_w, h2h_w, out_w):
    perm = np.concatenate([np.arange(0, 2 * H), np.arange(3 * H, 4 * H),
                           np.arange(2 * H, 3 * H)])  # [i,f,g,o] -> [i,f,o,g]
    wx2h = x2h_w[perm].T.astype(np.float32)            # [128, 1024]
    wh2h = h2h_w[perm].T.astype(np.float32)            # [256, 1024]
    wxp = (fc2_w @ fc1_w[:, :D]).T.astype(np.float32)  # [128, 16]
    whp = (fc2_w @ fc1_w[:, D:]).T.astype(np.float32)  # [256, 16]
    wout = out_w.T.astype(np.float32)                  # [256, 128]
    pack = np.zeros((128, W_END), np.float32)
    pack[:, W_X2H:W_X2H + 1024] = wx2h
    pack[:, W_H2H:W_H2H + 1024] = wh2h[0:128]
    pack[:, W_H2H + 1024:W_H2H + 2048] = wh2h[128:256]
    pack[:, W_XP:W_XP + 16] = wxp
    pack[:, W_HP:W_HP + 16] = whp[0:128]
    pack[:, W_HP + 16:W_HP + 32] = whp[128:256]
    pack[:, W_OUT:W_OUT + 128] = wout[0:128]
    pack[:, W_OUT + 128:W_OUT + 256] = wout[128:256]
    pack[:, W_IOTA:W_IOTA + 32] = np.arange(32, dtype=np.float32)[None, :]
    return pack


@functools.lru_cache(maxsize=1)
def _gumbel_all():
    import jax
    import jax.numpy as jnp
    with jax.default_device(jax.devices("cpu")[0]):
        keys = jax.random.split(jax.random.key(42), T)
        gs = [np.asarray(jax.random.gumbel(keys[jj], (B, A), jnp.float32))
              for jj in range(1, T)]
    return np.stack(gs)  # [255, B, 16]


_RUNNER = None


def _get_runner():
    global _RUNNER
    if _RUNNER is None:
        import jax
        from jax.sharding import Mesh, PartitionSpec
        from jax.experimental.shard_map import shard_map
        from concourse import bass2jax

        nc = build_bass()
        _split_sync_waits(nc)
        bass2jax.install_neuronx_cc_hook()
        partition_name = nc.partition_id_tensor.name if nc.partition_id_tensor else None
        in_names, out_names, out_avals, zero_outs = [], [], [], []
        for alloc in nc.m.functions[0].allocations:
            if not isinstance(alloc, mybir.MemoryLocationSet):
                continue
            name = alloc.memorylocations[0].name
            if alloc.kind == "ExternalInput":
                if name != partition_name:
                    in_names.append(name)
            elif alloc.kind == "ExternalOutput":
                out_names.append(name)
                shape = tuple(alloc.tensor_shape)
                dtype = mybir.dt.np(alloc.dtype)
                out_avals.append(jax.core.ShapedArray(shape, dtype))
                zero_outs.append(np.zeros(shape, dtype))
        n_params = len(in_names)
        n_outs = len(out_avals)
        in_names_all = in_names + out_names
        if partition_name is not None:
            in_names_all = in_names_all + [partition_name]
        donate = tuple(range(n_params, n_params + n_outs))

        def _body(*args):
            operands = list(args)
            if partition_name is not None:
                operands.append(bass2jax.partition_id_tensor())
            outs = bass2jax._bass_exec_p.bind(
                *operands, out_avals=tuple(out_avals), in_names=tuple(in_names_all),
                out_names=tuple(out_names), lowering_input_output_aliases=(),
                sim_require_finite=True, sim_require_nnan=True, nc=nc)
            return tuple(outs)

        devices = jax.devices()[:NCORES]
        mesh = Mesh(np.asarray(devices), ("core",))
        in_specs = (PartitionSpec("core"),) * (n_params + n_outs)
        out_specs = (PartitionSpec("core"),) * len(out_names)
        fn = jax.jit(shard_map(_body, mesh=mesh, in_specs=in_specs,
                               out_specs=out_specs, check_rep=False),
                     donate_argnums=donate, keep_unused=True)
        _RUNNER = (fn, in_names, out_names, out_avals, zero_outs)
    return _RUNNER


def kernel(input, fc1_w, fc1_b, fc2_w, fc2_b, x2h_w, x2h_b, h2h_w, h2h_b,
           out_w, out_b):
    input = np.asarray(input, np.float32)
    for bias in (fc1_b, fc2_b, x2h_b, h2h_b, out_b):
        assert np.all(np.asarray(bias) == 0.0), "kernel assumes zero biases"
    wpack = _prep_weights(np.asarray(fc1_w, np.float32), np.asarray(fc2_w, np.float32),
                          np.asarray(x2h_w, np.float32), np.asarray(h2h_w, np.float32),
                          np.asarray(out_w, np.float32))
    G = _gumbel_all()  # [255, B, 16]

    fn, in_names, out_names, out_avals, zero_outs = _get_runner()

    per_core = {"XT": [], "GUM": [], "WPK": []}
    for c in range(NCORES):
        rows = slice(c * BL, (c + 1) * BL)
        per_core["XT"].append(np.ascontiguousarray(input[rows].transpose(1, 2, 0)))
        per_core["GUM"].append(
            np.ascontiguousarray(G[:, rows, :].transpose(1, 0, 2).reshape(BL, -1)))
        per_core["WPK"].append(wpack)
    ins = [np.concatenate(per_core[name], axis=0) for name in in_names]
    zeros = [np.zeros((NCORES * z.shape[0], *z.shape[1:]), z.dtype) for z in zero_outs]
    out_arrs = fn(*ins, *zeros)
    import jax
    jax.block_until_ready(out_arrs)
    outs = {name: np.asarray(out_arrs[i]) for i, name in enumerate(out_names)}

    outf = outs["OUTF"].reshape(NCORES, BL, O + NSTEP)
    outi = outs["OUTI"].reshape(NCORES, BL, NSTEP)
    opt = outf[:, :, 0:O].reshape(B, O)
    probs = outf[:, :, O:].reshape(B, NSTEP).T.copy()
    actions = outi.reshape(B, NSTEP).T.astype(np.int32).copy()
    return opt, actions, probs


# revision 8
# speedup vs baseline: 23.1473x; 23.1473x over previous
"""Trainium2 Bass kernel for nn_DynamicLSTM (scatter_memory).

Self-contained: kernel(**inputs) -> (opt, agent_action, agent_prob).

Strategy: data-parallel over batch (B=1024) across 8 NeuronCores, 128 rows
per core (= SBUF partitions). The T=255 recurrent scan runs fully unrolled
on-device. Per step:
  - policy logits = x_t @ Wx'.T + h @ Wh'.T  (Wx' = fc2@fc1_x, Wh' = fc2@fc1_h,
    folded on host; the two-layer policy MLP has no nonlinearity between)
  - + host-precomputed Gumbel noise (exactly jax.random.categorical's),
    argmax -> action
  - rolling 16-slot ring of (c, h) states in SBUF; per-row gather by action
    implemented as a 16-slot mask-multiply-accumulate on the vector engine
  - LSTM cell with gates from fp32 PE matmuls (x2h moving-weights with x^T
    stationary; h2h with gathered-h^T stationary via PE transpose)
  - log-prob computation deferred and batched per chunk.
"""
import functools
import time

import numpy as np

import concourse.bass as bass
import concourse.mybir as mybir
import concourse.tile as tile
from concourse.vector_clock import ScopedClock, VectorClock

F32 = mybir.dt.float32
I32 = mybir.dt.int32
U32 = mybir.dt.uint32

B, T, D, H, U, A, O = 1024, 256, 128, 256, 64, 16, 128
NCORES = 8
BL = B // NCORES          # 128 rows per core
NSTEP = T - 1             # 255 scan steps
CH = 64                   # chunk size for x/gumbel streaming + logp batching
AF = mybir.ActivationFunctionType
AL = mybir.AluOpType

# weight pack layout (f32 elements per partition)
W_X2H = 0                 # [128, 1024] x2h moving (d-part)
W_H2H = 1024              # [128, 2*1024] h2h moving (h%128-part, k-tile major)
W_XP = W_H2H + 2048       # [128, 16] policy x weights
W_HP = W_XP + 16          # [128, 2*16] policy h weights (k-tile major)
W_OUT = W_HP + 32         # [128, 2*128] out_w moving (k-tile major)
W_IOTA = W_OUT + 256      # [128, 32] iota 0..31
W_END = W_IOTA + 32


def _split_sync_waits(nc, max_waits=1):
    """Walrus (this build) rejects instructions carrying more than one
    sync-wait command; move excess waits onto same-engine nops."""
    ctr = 0
    for f in nc.m.functions:
        for blk in f.blocks:
            out = []
            for inst in blk.instructions:
                si = inst.sync_info
                if si is not None and len(si.on_wait) > max_waits:
                    waits = list(si.on_wait)
                    head, tail = waits[:-max_waits], waits[-max_waits:]
                    for w in head:
                        nop = mybir.InstNoOp(name=f"waitsplit_{ctr}", ins=[], outs=[])
                        ctr += 1
                        nop.engine = inst.engine
                        nop.sync_info = mybir.SyncInfo(on_wait=[w], on_update=[])
                        out.append(nop)
                    inst.sync_info = mybir.SyncInfo(
                        on_wait=tail, on_update=list(si.on_update))
                out.append(inst)
            blk.instructions[:] = out
    return ctr


class _ChunkedDrainTC(tile.TileContext):
    """Chunk kernel-tail drain waits over many sync nops (walrus limits the
    number of sync-wait commands a single instruction may carry)."""

    def _drain_and_barrier(self, tick_clock, wait_clock):
        import re
        ticks = [int(x) for x in re.findall(r"-?\d+", repr(tick_clock.global_clock))]
        for p, t in enumerate(ticks):
            if t > 0:
                sub = [0] * len(ticks)
                sub[p] = t
                nop = self.nc.sync.nop()
                wait_clock.add_sem_waits(nop.ins, ScopedClock({None: VectorClock(sub)}))
        self.nc.sync.drain()
        self.nc.all_engine_barrier()
        assert self.sems is not None
        popped = self.nc._tile_sem_poison_stack.pop()
        assert popped is self._sem_poison
        self.nc.clear_and_free_semaphores(list(self.sems.allocated().values()))
        self.nc.all_engine_barrier()


def build_bass(nsteps=NSTEP):
    nc = bass.Bass("TRN2", target_bir_lowering=False, debug=False)
    nx = nsteps + 1  # number of x timesteps used (prologue + nsteps)

    XT = nc.dram_tensor("XT", [nx, D, BL], F32, kind="ExternalInput").ap()
    GUM = nc.dram_tensor("GUM", [BL, nsteps * A], F32, kind="ExternalInput").ap()
    WPK = nc.dram_tensor("WPK", [128, W_END], F32, kind="ExternalInput").ap()
    OUTF = nc.dram_tensor("OUTF", [BL, O + nsteps], F32, kind="ExternalOutput").ap()
    OUTI = nc.dram_tensor("OUTI", [BL, nsteps], I32, kind="ExternalOutput").ap()

    def sb(name, shape, dt=F32):
        return nc.alloc_sbuf_tensor(name, shape, dt).ap()

    def ps(name, shape):
        return nc.alloc_psum_tensor(name, shape, F32).ap()

    with _ChunkedDrainTC(nc) as tc:
        W = sb("W", [128, W_END])
        # x chunks (double buffered), t-slice layout [d-part, b]
        nchunk = (nx + CH - 1) // CH
        xt_buf = [sb(f"xt{i}", [128, CH * BL]) for i in range(2)]
        gm_buf = [sb(f"gm{i}", [128, CH * A]) for i in range(2)]
        RING = sb("RING", [128, 16 * 2 * H])      # per slot: c(256) | h(256)
        S = sb("S", [128, 1536])                  # sig(768) | tg(256) | wc(256) | wh(256)
        P = sb("P", [128, 512])
        A1 = sb("A1", [128, 512])
        whT = sb("whT", [128, 256])
        hT = sb("hT", [128, 256])
        TC = sb("TCt", [128, H])
        LG = sb("LG", [128, A])
        mx = sb("mx", [128, 8])
        mi = sb("mi", [128, 8], U32)
        m32 = sb("m32", [128, 32])
        m16 = sb("m16", [128, 16])
        lstore = sb("lstore", [128, nsteps * A])
        act_f = sb("act_f", [128, nsteps])
        act_i = sb("act_i", [128, nsteps], I32)
        probs = sb("probs", [128, nsteps])
        # logp batch scratch
        sc0 = sb("sc0", [128, CH * A])
        sc1 = sb("sc1", [128, CH * A])
        r0 = sb("r0", [128, CH])
        r1 = sb("r1", [128, CH])
        r2 = sb("r2", [128, CH])
        opt_t = sb("opt_t", [128, O])
        eps = sb("eps", [128, 1])

        g_ps = ps("g_ps", [128, 1024])
        l_ps = ps("l_ps", [128, 16])
        t_ps = ps("t_ps", [128, 256])
        o_ps = ps("o_ps", [128, 128])

        ident = sb("ident", [128, 128])
        from concourse.masks import make_identity
        make_identity(nc, ident)

        nc.sync.dma_start(W, WPK)
        iota32 = W[:, W_IOTA:W_IOTA + 32]

        def load_chunk(c):
            i0 = c * CH
            ilen = min(CH, nx - i0)
            nc.sync.dma_start(
                xt_buf[c % 2][:, 0:ilen * BL].rearrange("p (t b) -> p t b", t=ilen),
                XT[i0:i0 + ilen].rearrange("t d b -> d t b"))
            j0 = c * CH
            jlen = min(CH, nsteps - j0)
            if jlen > 0:
                nc.sync.dma_start(gm_buf[c % 2][:, 0:jlen * A], GUM[:, j0 * A:(j0 + jlen) * A])

        load_chunk(0)
        nc.vector.memset(RING[:], 0.0)

        def xt_at(i):
            return xt_buf[(i // CH) % 2][:, (i % CH) * BL:(i % CH + 1) * BL]

        def gum_at(j):
            return gm_buf[(j // CH) % 2][:, (j % CH) * A:(j % CH + 1) * A]

        def transpose_256(src_ap, dst_sbuf):
            # src [128, 256] -> dst [128, 2*128] (k-tile major transpose)
            nc.tensor.transpose(t_ps[:, 0:128], src_ap[:, 0:128], ident[:])
            nc.tensor.transpose(t_ps[:, 128:256], src_ap[:, 128:256], ident[:])
            nc.vector.tensor_copy(dst_sbuf[:], t_ps[:])

        def cell(j, with_h2h):
            """gates for step j are in g_ps; produce s_{j+1} in RING."""
            snew = (j + 1) % 16
            nc.scalar.activation(S[:, 0:768], g_ps[:, 0:768], AF.Sigmoid)
            nc.scalar.activation(S[:, 768:1024], g_ps[:, 768:1024], AF.Tanh)
            if with_h2h:
                # P = [sig_i, sig_f] * [tg, wc]
                nc.vector.tensor_tensor(P[:], S[:, 0:512], S[:, 768:1280], AL.mult)
                nc.vector.tensor_tensor(
                    RING[:, snew * 512:snew * 512 + 256], P[:, 0:256], P[:, 256:512], AL.add)
            else:
                # prologue: c = sig_i * tg
                nc.vector.tensor_tensor(
                    RING[:, snew * 512:snew * 512 + 256], S[:, 0:256], S[:, 768:1024], AL.mult)
            nc.scalar.activation(TC[:], RING[:, snew * 512:snew * 512 + 256], AF.Tanh)
            nc.vector.tensor_tensor(
                RING[:, snew * 512 + 256:snew * 512 + 512], S[:, 512:768], TC[:], AL.mult)
            # hT for the next step's policy (and the final output head)
            transpose_256(RING[:, snew * 512 + 256:snew * 512 + 512], hT)

        # ---- prologue: s_0 = cell(x_0, 0, 0) -> RING slot 0
        x0 = xt_at(0)
        nc.tensor.matmul(g_ps[:, 0:512], lhsT=x0, rhs=W[:, W_X2H:W_X2H + 512],
                         start=True, stop=True)
        nc.tensor.matmul(g_ps[:, 512:1024], lhsT=x0, rhs=W[:, W_X2H + 512:W_X2H + 1024],
                         start=True, stop=True)
        cell(-1, with_h2h=False)

        # ---- scan
        for j in range(nsteps):
            xi = j + 1
            if xi % CH == 0 and xi // CH < nchunk:
                load_chunk(xi // CH)
            if xi % CH == 1 and xi // CH + 1 < nchunk:
                load_chunk(xi // CH + 1)
            xt = xt_at(xi)
            # policy logits: x part + h part (hT holds h_j)
            nc.tensor.matmul(l_ps[:], lhsT=xt, rhs=W[:, W_XP:W_XP + 16],
                             start=True, stop=False)
            nc.tensor.matmul(l_ps[:], lhsT=hT[:, 0:128],
                             rhs=W[:, W_HP:W_HP + 16], start=False, stop=False)
            nc.tensor.matmul(l_ps[:], lhsT=hT[:, 128:256],
                             rhs=W[:, W_HP + 16:W_HP + 32], start=False, stop=True)
            # gates: x2h early (accumulated in g_ps banks)
            nc.tensor.matmul(g_ps[:, 0:512], lhsT=xt, rhs=W[:, W_X2H:W_X2H + 512],
                             start=True, stop=False)
            nc.tensor.matmul(g_ps[:, 512:1024], lhsT=xt,
                             rhs=W[:, W_X2H + 512:W_X2H + 1024], start=True, stop=False)
            # sampling
            nc.vector.scalar_tensor_tensor(LG[:], l_ps[:], 0.0, gum_at(j),
                                           AL.add, AL.add)
            nc.vector.tensor_copy(lstore[:, j * A:(j + 1) * A], l_ps[:])
            nc.vector.max(mx[:], LG[:])
            nc.vector.max_index(mi[:], mx[:], LG[:])
            nc.vector.tensor_copy(act_f[:, j:j + 1], mi[:, 0:1])
            nc.vector.tensor_copy(act_i[:, j:j + 1], mi[:, 0:1])
            # ring-slot one-hot over 32 then fold mod 16
            cj = float((j + 1) % 16)
            nc.vector.tensor_scalar(m32[:], iota32, act_f[:, j:j + 1], cj,
                                    AL.subtract, AL.is_equal)
            nc.vector.tensor_tensor(m16[:], m32[:, 0:16], m32[:, 16:32], AL.add)
            # gather: 16-slot mask-multiply-accumulate -> S[:, 1024:1536]
            nc.vector.tensor_scalar(A1[:], RING[:, 0:512], m16[:, 0:1], None, AL.mult)
            for r in range(1, 16):
                dst = S[:, 1024:1536] if r % 2 == 1 else A1[:]
                src = A1[:] if r % 2 == 1 else S[:, 1024:1536]
                nc.vector.scalar_tensor_tensor(
                    dst, RING[:, r * 512:(r + 1) * 512], m16[:, r:r + 1], src,
                    AL.mult, AL.add)
            # wc||wh landed in S[:, 1024:1536] (r=15 odd)
            transpose_256(S[:, 1280:1536], whT)
            # h2h accumulate into gates
            nc.tensor.matmul(g_ps[:, 0:512], lhsT=whT[:, 0:128],
                             rhs=W[:, W_H2H:W_H2H + 512], start=False, stop=False)
            nc.tensor.matmul(g_ps[:, 512:1024], lhsT=whT[:, 0:128],
                             rhs=W[:, W_H2H + 512:W_H2H + 1024], start=False, stop=False)
            nc.tensor.matmul(g_ps[:, 0:512], lhsT=whT[:, 128:256],
                             rhs=W[:, W_H2H + 1024:W_H2H + 1536], start=False, stop=True)
            nc.tensor.matmul(g_ps[:, 512:1024], lhsT=whT[:, 128:256],
                             rhs=W[:, W_H2H + 1536:W_H2H + 2048], start=False, stop=True)
            cell(j, with_h2h=True)

            # ---- deferred logp for a finished chunk
            if (j + 1) % CH == 0 or j == nsteps - 1:
                c0 = (j // CH) * CH
                n = j + 1 - c0
                L = lstore[:, c0 * A:(j + 1) * A]
                L3 = L.rearrange("p (t a) -> p t a", a=A)
                nc.vector.tensor_reduce(r0[:, 0:n], L3, mybir.AxisListType.X, AL.max)
                nc.vector.tensor_tensor(
                    sc0[:, 0:n * A].rearrange("p (t a) -> p t a", a=A), L3,
                    r0[:, 0:n].unsqueeze(2).broadcast_to([128, n, A]), AL.subtract)
                nc.scalar.activation(sc1[:, 0:n * A], sc0[:, 0:n * A], AF.Exp)
                nc.vector.tensor_reduce(
                    r1[:, 0:n], sc1[:, 0:n * A].rearrange("p (t a) -> p t a", a=A),
                    mybir.AxisListType.X, AL.add)
                nc.scalar.activation(r1[:, 0:n], r1[:, 0:n], AF.Ln)
                # mask of chosen actions; sc0 still holds (L - rmax)
                nc.vector.tensor_tensor(
                    sc1[:, 0:n * A].rearrange("p (t a) -> p t a", a=A),
                    W[:, W_IOTA:W_IOTA + 16].unsqueeze(1).broadcast_to([128, n, A]),
                    act_f[:, c0:j + 1].unsqueeze(2).broadcast_to([128, n, A]),
                    AL.is_equal)
                nc.vector.tensor_tensor(sc0[:, 0:n * A], sc0[:, 0:n * A],
                                        sc1[:, 0:n * A], AL.mult)
                nc.vector.tensor_reduce(
                    r2[:, 0:n], sc0[:, 0:n * A].rearrange("p (t a) -> p t a", a=A),
                    mybir.AxisListType.X, AL.add)
                # logp = (sel - rmax...) wait: sel already = logit[a] - rmax
                nc.vector.tensor_tensor(probs[:, c0:j + 1], r2[:, 0:n], r1[:, 0:n],
                                        AL.subtract)

        # ---- epilogue: opt = softmax(h_last @ out_w.T); hT holds h_last^T
        nc.tensor.matmul(o_ps[:], lhsT=hT[:, 0:128], rhs=W[:, W_OUT:W_OUT + 128],
                         start=True, stop=False)
        nc.tensor.matmul(o_ps[:], lhsT=hT[:, 128:256], rhs=W[:, W_OUT + 128:W_OUT + 256],
                         start=False, stop=True)
        nc.vector.tensor_reduce(eps[:], o_ps[:], mybir.AxisListType.X, AL.max)
        nc.vector.tensor_scalar(eps[:], eps[:], -1.0, None, AL.mult)
        nc.scalar.activation(opt_t[:], o_ps[:], AF.Exp, bias=eps[:])
        nc.vector.tensor_reduce(eps[:], opt_t[:], mybir.AxisListType.X, AL.add)
        nc.vector.reciprocal(eps[:], eps[:])
        nc.vector.tensor_scalar(opt_t[:], opt_t[:], eps[:], None, AL.mult)

        nc.sync.dma_start(OUTF[:, 0:O], opt_t[:])
        nc.sync.dma_start(OUTF[:, O:O + nsteps], probs[:])
        nc.sync.dma_start(OUTI, act_i[:])
    return nc


# ---------------------------------------------------------------- host side

def _prep_weights(fc1_w, fc2_w, x2h_w, h2h_w, out_w):
    perm = np.concatenate([np.arange(0, 2 * H), np.arange(3 * H, 4 * H),
                           np.arange(2 * H, 3 * H)])  # [i,f,g,o] -> [i,f,o,g]
    wx2h = x2h_w[perm].T.astype(np.float32)            # [128, 1024]
    wh2h = h2h_w[perm].T.astype(np.float32)            # [256, 1024]
    wxp = (fc2_w @ fc1_w[:, :D]).T.astype(np.float32)  # [128, 16]
    whp = (fc2_w @ fc1_w[:, D:]).T.astype(np.float32)  # [256, 16]
    wout = out_w.T.astype(np.float32)                  # [256, 128]
    pack = np.zeros((128, W_END), np.float32)
    pack[:, W_X2H:W_X2H + 1024] = wx2h
    pack[:, W_H2H:W_H2H + 1024] = wh2h[0:128]
    pack[:, W_H2H + 1024:W_H2H + 2048] = wh2h[128:256]
    pack[:, W_XP:W_XP + 16] = wxp
    pack[:, W_HP:W_HP + 16] = whp[0:128]
    pack[:, W_HP + 16:W_HP + 32] = whp[128:256]
    pack[:, W_OUT:W_OUT + 128] = wout[0:128]
    pack[:, W_OUT + 128:W_OUT + 256] = wout[128:256]
    pack[:, W_IOTA:W_IOTA + 32] = np.arange(32, dtype=np.float32)[None, :]
    return pack


@functools.lru_cache(maxsize=1)
def _gumbel_all():
    import jax
    import jax.numpy as jnp
    with jax.default_device(jax.devices("cpu")[0]):
        keys = jax.random.split(jax.random.key(42), T)
        gs = [np.asarray(jax.random.gumbel(keys[jj], (B, A), jnp.float32))
              for jj in range(1, T)]
    return np.stack(gs)  # [255, B, 16]


_RUNNER = None


def _get_runner():
    global _RUNNER
    if _RUNNER is None:
        import jax
        from jax.sharding import Mesh, PartitionSpec
        from jax.experimental.shard_map import shard_map
        from concourse import bass2jax

        nc = build_bass()
        _split_sync_waits(nc)
        bass2jax.install_neuronx_cc_hook()
        partition_name = nc.partition_id_tensor.name if nc.partition_id_tensor else None
        in_names, out_names, out_avals, zero_outs = [], [], [], []
        for alloc in nc.m.functions[0].allocations:
            if not isinstance(alloc, mybir.MemoryLocationSet):
                continue
            name = alloc.memorylocations[0].name
            if alloc.kind == "ExternalInput":
                if name != partition_name:
                    in_names.append(name)
            elif alloc.kind == "ExternalOutput":
                out_names.append(name)
                shape = tuple(alloc.tensor_shape)
                dtype = mybir.dt.np(alloc.dtype)
                out_avals.append(jax.core.ShapedArray(shape, dtype))
                zero_outs.append(np.zeros(shape, dtype))
        n_params = len(in_names)
        n_outs = len(out_avals)
        in_names_all = in_names + out_names
        if partition_name is not None:
            in_names_all = in_names_all + [partition_name]
        donate = tuple(range(n_params, n_params + n_outs))

        def _body(*args):
            operands = list(args)
            if partition_name is not None:
                operands.append(bass2jax.partition_id_tensor())
            outs = bass2jax._bass_exec_p.bind(
                *operands, out_avals=tuple(out_avals), in_names=tuple(in_names_all),
                out_names=tuple(out_names), lowering_input_output_aliases=(),
                sim_require_finite=True, sim_require_nnan=True, nc=nc)
            return tuple(outs)

        devices = jax.devices()[:NCORES]
        mesh = Mesh(np.asarray(devices), ("core",))
        in_specs = (PartitionSpec("core"),) * (n_params + n_outs)
        out_specs = (PartitionSpec("core"),) * len(out_names)
        fn = jax.jit(shard_map(_body, mesh=mesh, in_specs=in_specs,
                               out_specs=out_specs, check_rep=False),
                     donate_argnums=donate, keep_unused=True)
        _RUNNER = (fn, in_names, out_names, out_avals, zero_outs)
    return _RUNNER


def _prep_ins(input, fc1_w, fc2_w, x2h_w, h2h_w, out_w):
    wpack = _prep_weights(np.asarray(fc1_w, np.float32), np.asarray(fc2_w, np.float32),
                          np.asarray(x2h_w, np.float32), np.asarray(h2h_w, np.float32),
                          np.asarray(out_w, np.float32))
    G = _gumbel_all()
    per_core = {"XT": [], "GUM": [], "WPK": []}
    for c in range(NCORES):
        rows = slice(c * BL, (c + 1) * BL)
        per_core["XT"].append(np.ascontiguousarray(input[rows].transpose(1, 2, 0)))
        per_core["GUM"].append(
            np.ascontiguousarray(G[:, rows, :].transpose(1, 0, 2).reshape(BL, -1)))
        per_core["WPK"].append(wpack)
    return per_core


def kernel_timed(input, fc1_w, fc1_b, fc2_w, fc2_b, x2h_w, x2h_b, h2h_w, h2h_b,
                 out_w, out_b, n_runs=5):
    """Run with device-resident inputs; returns (outputs, median_exec_seconds)."""
    import jax
    from jax.sharding import NamedSharding, PartitionSpec
    input = np.asarray(input, np.float32)
    per_core = _prep_ins(input, fc1_w, fc2_w, x2h_w, h2h_w, out_w)
    fn, in_names, out_names, out_avals, zero_outs = _get_runner()
    ins_np = [np.concatenate(per_core[name], axis=0) for name in in_names]
    mesh = fn._mesh if hasattr(fn, "_mesh") else None
    from jax.sharding import Mesh
    devices = jax.devices()[:NCORES]
    mesh = Mesh(np.asarray(devices), ("core",))
    sh = NamedSharding(mesh, PartitionSpec("core"))
    ins_dev = [jax.device_put(a, sh) for a in ins_np]
    jax.block_until_ready(ins_dev)

    def once():
        zeros = [np.zeros((NCORES * z.shape[0], *z.shape[1:]), z.dtype)
                 for z in zero_outs]
        t0 = time.time()
        out_arrs = fn(*ins_dev, *zeros)
        jax.block_until_ready(out_arrs)
        return time.time() - t0, out_arrs

    once()  # warm
    times = []
    out_arrs = None
    for _ in range(n_runs):
        dt, out_arrs = once()
        times.append(dt)
    med = sorted(times)[len(times) // 2]
    outs = {name: np.asarray(out_arrs[i]) for i, name in enumerate(out_names)}
    outf = outs["OUTF"].reshape(NCORES, BL, O + NSTEP)
    outi = outs["OUTI"].reshape(NCORES, BL, NSTEP)
    opt = outf[:, :, 0:O].reshape(B, O)
    probs = outf[:, :, O:].reshape(B, NSTEP).T.copy()
    actions = outi.reshape(B, NSTEP).T.astype(np.int32).copy()
    return (opt, actions, probs), med, times


def kernel(input, fc1_w, fc1_b, fc2_w, fc2_b, x2h_w, x2h_b, h2h_w, h2h_b,
           out_w, out_b):
    input = np.asarray(input, np.float32)
    for bias in (fc1_b, fc2_b, x2h_b, h2h_b, out_b):
        assert np.all(np.asarray(bias) == 0.0), "kernel assumes zero biases"
    wpack = _prep_weights(np.asarray(fc1_w, np.float32), np.asarray(fc2_w, np.float32),
                          np.asarray(x2h_w, np.float32), np.asarray(h2h_w, np.float32),
                          np.asarray(out_w, np.float32))
    G = _gumbel_all()  # [255, B, 16]

    fn, in_names, out_names, out_avals, zero_outs = _get_runner()

    per_core = {"XT": [], "GUM": [], "WPK": []}
    for c in range(NCORES):
        rows = slice(c * BL, (c + 1) * BL)
        per_core["XT"].append(np.ascontiguousarray(input[rows].transpose(1, 2, 0)))
        per_core["GUM"].append(
            np.ascontiguousarray(G[:, rows, :].transpose(1, 0, 2).reshape(BL, -1)))
        per_core["WPK"].append(wpack)
    ins = [np.concatenate(per_core[name], axis=0) for name in in_names]
    zeros = [np.zeros((NCORES * z.shape[0], *z.shape[1:]), z.dtype) for z in zero_outs]
    out_arrs = fn(*ins, *zeros)
    import jax
    jax.block_until_ready(out_arrs)
    outs = {name: np.asarray(out_arrs[i]) for i, name in enumerate(out_names)}

    outf = outs["OUTF"].reshape(NCORES, BL, O + NSTEP)
    outi = outs["OUTI"].reshape(NCORES, BL, NSTEP)
    opt = outf[:, :, 0:O].reshape(B, O)
    probs = outf[:, :, O:].reshape(B, NSTEP).T.copy()
    actions = outi.reshape(B, NSTEP).T.astype(np.int32).copy()
    return opt, actions, probs


# revision 12
# speedup vs baseline: 27.4351x; 1.1852x over previous
"""Trainium2 Bass kernel for nn_DynamicLSTM (scatter_memory).

Self-contained: kernel(**inputs) -> (opt, agent_action, agent_prob).

Strategy: data-parallel over batch (B=1024) across 8 NeuronCores, 128 rows
per core (= SBUF partitions). The T=255 recurrent scan runs fully unrolled
on-device. Per step:
  - policy logits = x_t @ Wx'.T + h @ Wh'.T  (Wx' = fc2@fc1_x, Wh' = fc2@fc1_h,
    folded on host; the two-layer policy MLP has no nonlinearity between)
  - + host-precomputed Gumbel noise (exactly jax.random.categorical's),
    argmax -> action
  - rolling 16-slot ring of (c, h) states in SBUF; per-row gather by action
    implemented as a 16-slot mask-multiply-accumulate on the vector engine
  - LSTM cell with gates from fp32 PE matmuls (x2h moving-weights with x^T
    stationary; h2h with gathered-h^T stationary via PE transpose)
  - log-prob computation deferred and batched per chunk.
"""
import functools
import time

import numpy as np

import concourse.bass as bass
import concourse.mybir as mybir
import concourse.tile as tile
from concourse.vector_clock import ScopedClock, VectorClock

F32 = mybir.dt.float32
I32 = mybir.dt.int32
U32 = mybir.dt.uint32

B, T, D, H, U, A, O = 1024, 256, 128, 256, 64, 16, 128
NCORES = 8
BL = B // NCORES          # 128 rows per core
NSTEP = T - 1             # 255 scan steps
CH = 64                   # chunk size for x/gumbel streaming + logp batching
AF = mybir.ActivationFunctionType
AL = mybir.AluOpType

# weight pack layout (f32 elements per partition)
W_X2H = 0                 # [128, 1024] x2h moving (d-part)
W_H2H = 1024              # [128, 2*1024] h2h moving (h%128-part, k-tile major)
W_XP = W_H2H + 2048       # [128, 16] policy x weights
W_HP = W_XP + 16          # [128, 2*16] policy h weights (k-tile major)
W_OUT = W_HP + 32         # [128, 2*128] out_w moving (k-tile major)
W_IOTA = W_OUT + 256      # [128, 32] iota 0..31
W_END = W_IOTA + 32


def _split_sync_waits(nc, max_waits=1):
    """Walrus (this build) rejects instructions carrying more than one
    sync-wait command; move excess waits onto same-engine nops."""
    ctr = 0
    for f in nc.m.functions:
        for blk in f.blocks:
            out = []
            for inst in blk.instructions:
                si = inst.sync_info
                if si is not None and len(si.on_wait) > max_waits:
                    waits = list(si.on_wait)
                    head, tail = waits[:-max_waits], waits[-max_waits:]
                    for w in head:
                        nop = mybir.InstNoOp(name=f"waitsplit_{ctr}", ins=[], outs=[])
                        ctr += 1
                        nop.engine = inst.engine
                        nop.sync_info = mybir.SyncInfo(on_wait=[w], on_update=[])
                        out.append(nop)
                    inst.sync_info = mybir.SyncInfo(
                        on_wait=tail, on_update=list(si.on_update))
                out.append(inst)
            blk.instructions[:] = out
    return ctr


class _ChunkedDrainTC(tile.TileContext):
    """Chunk kernel-tail drain waits over many sync nops (walrus limits the
    number of sync-wait commands a single instruction may carry)."""

    def _drain_and_barrier(self, tick_clock, wait_clock):
        import re
        ticks = [int(x) for x in re.findall(r"-?\d+", repr(tick_clock.global_clock))]
        for p, t in enumerate(ticks):
            if t > 0:
                sub = [0] * len(ticks)
                sub[p] = t
                nop = self.nc.sync.nop()
                wait_clock.add_sem_waits(nop.ins, ScopedClock({None: VectorClock(sub)}))
        self.nc.sync.drain()
        self.nc.all_engine_barrier()
        assert self.sems is not None
        popped = self.nc._tile_sem_poison_stack.pop()
        assert popped is self._sem_poison
        self.nc.clear_and_free_semaphores(list(self.sems.allocated().values()))
        self.nc.all_engine_barrier()


def build_bass(nsteps=NSTEP):
    nc = bass.Bass("TRN2", target_bir_lowering=False, debug=False)
    nx = nsteps + 1  # number of x timesteps used (prologue + nsteps)

    XT = nc.dram_tensor("XT", [nx, D, BL], F32, kind="ExternalInput").ap()
    GUM = nc.dram_tensor("GUM", [BL, nsteps * A], F32, kind="ExternalInput").ap()
    WPK = nc.dram_tensor("WPK", [128, W_END], F32, kind="ExternalInput").ap()
    OUTF = nc.dram_tensor("OUTF", [BL, O + nsteps], F32, kind="ExternalOutput").ap()
    OUTI = nc.dram_tensor("OUTI", [BL, nsteps], I32, kind="ExternalOutput").ap()

    def sb(name, shape, dt=F32):
        return nc.alloc_sbuf_tensor(name, shape, dt).ap()

    def ps(name, shape):
        return nc.alloc_psum_tensor(name, shape, F32).ap()

    with _ChunkedDrainTC(nc) as tc:
        W = sb("W", [128, W_END])
        # x chunks (double buffered), t-slice layout [d-part, b]
        nchunk = (nx + CH - 1) // CH
        xt_buf = [sb(f"xt{i}", [128, CH * BL]) for i in range(2)]
        gm_buf = [sb(f"gm{i}", [128, CH * A]) for i in range(2)]
        RING = sb("RING", [128, 16 * 2 * H])      # per slot: c(256) | h(256)
        S = sb("S", [128, 1536])                  # sig(768) | tg(256) | wc(256) | wh(256)
        P = sb("P", [128, 512])
        A1 = sb("A1", [128, 512])
        G1 = sb("G1", [128, 512])
        G2 = sb("G2", [128, 512])
        G3 = sb("G3", [128, 512])
        whT = sb("whT", [128, 256])
        hT = sb("hT", [128, 256])
        TC = sb("TCt", [128, H])
        LG = sb("LG", [128, A])
        mx = sb("mx", [128, 8])
        mi = sb("mi", [128, 8], U32)
        m32 = sb("m32", [128, 32])
        m16 = sb("m16", [128, 16])
        lstore = sb("lstore", [128, nsteps * A])
        act_f = sb("act_f", [128, nsteps])
        act_i = sb("act_i", [128, nsteps], I32)
        probs = sb("probs", [128, nsteps])
        # logp batch scratch
        sc0 = sb("sc0", [128, CH * A])
        sc1 = sb("sc1", [128, CH * A])
        r0 = sb("r0", [128, CH])
        r1 = sb("r1", [128, CH])
        r2 = sb("r2", [128, CH])
        opt_t = sb("opt_t", [128, O])
        eps = sb("eps", [128, 1])

        g_ps2 = [ps("g_ps0", [128, 1024]), ps("g_ps1", [128, 1024])]
        l_ps2 = [ps("l_ps0", [128, 16]), ps("l_ps1", [128, 16])]
        t_ps = ps("t_ps", [128, 256])
        o_ps = ps("o_ps", [128, 128])

        ident = sb("ident", [128, 128])
        from concourse.masks import make_identity
        make_identity(nc, ident)

        nc.sync.dma_start(W, WPK)
        iota32 = W[:, W_IOTA:W_IOTA + 32]

        def load_chunk(c):
            i0 = c * CH
            ilen = min(CH, nx - i0)
            nc.sync.dma_start(
                xt_buf[c % 2][:, 0:ilen * BL].rearrange("p (t b) -> p t b", t=ilen),
                XT[i0:i0 + ilen].rearrange("t d b -> d t b"))
            j0 = c * CH
            jlen = min(CH, nsteps - j0)
            if jlen > 0:
                nc.sync.dma_start(gm_buf[c % 2][:, 0:jlen * A], GUM[:, j0 * A:(j0 + jlen) * A])

        load_chunk(0)
        nc.vector.memset(RING[:], 0.0)

        def xt_at(i):
            return xt_buf[(i // CH) % 2][:, (i % CH) * BL:(i % CH + 1) * BL]

        def gum_at(j):
            return gm_buf[(j // CH) % 2][:, (j % CH) * A:(j % CH + 1) * A]

        def transpose_256(src_ap, dst_sbuf):
            # src [128, 256] -> dst [128, 2*128] (k-tile major transpose)
            nc.tensor.transpose(t_ps[:, 0:128], src_ap[:, 0:128], ident[:])
            nc.tensor.transpose(t_ps[:, 128:256], src_ap[:, 128:256], ident[:])
            nc.vector.tensor_copy(dst_sbuf[:], t_ps[:])

        def cell(j, with_h2h, g_ps):
            """gates for step j are in g_ps; produce s_{j+1} in RING."""
            snew = (j + 1) % 16
            nc.scalar.activation(S[:, 0:768], g_ps[:, 0:768], AF.Sigmoid)
            nc.scalar.activation(S[:, 768:1024], g_ps[:, 768:1024], AF.Tanh)
            if with_h2h:
                # P = [sig_i, sig_f] * [tg, wc]
                nc.vector.tensor_tensor(P[:], S[:, 0:512], S[:, 768:1280], AL.mult)
                nc.vector.tensor_tensor(
                    RING[:, snew * 512:snew * 512 + 256], P[:, 0:256], P[:, 256:512], AL.add)
            else:
                # prologue: c = sig_i * tg
                nc.vector.tensor_tensor(
                    RING[:, snew * 512:snew * 512 + 256], S[:, 0:256], S[:, 768:1024], AL.mult)
            nc.scalar.activation(TC[:], RING[:, snew * 512:snew * 512 + 256], AF.Tanh)
            nc.vector.tensor_tensor(
                RING[:, snew * 512 + 256:snew * 512 + 512], S[:, 512:768], TC[:], AL.mult)
            # hT for the next step's policy (and the final output head)
            transpose_256(RING[:, snew * 512 + 256:snew * 512 + 512], hT)

        # ---- prologue: s_0 = cell(x_0, 0, 0) -> RING slot 0
        x0 = xt_at(0)
        g_pro = g_ps2[1]
        nc.tensor.matmul(g_pro[:, 0:512], lhsT=x0, rhs=W[:, W_X2H:W_X2H + 512],
                         start=True, stop=True)
        nc.tensor.matmul(g_pro[:, 512:1024], lhsT=x0, rhs=W[:, W_X2H + 512:W_X2H + 1024],
                         start=True, stop=True)
        cell(-1, with_h2h=False, g_ps=g_pro)

        # ---- scan
        for j in range(nsteps):
            xi = j + 1
            g_ps = g_ps2[j % 2]
            l_ps = l_ps2[j % 2]
            if xi % CH == 0 and xi // CH < nchunk:
                load_chunk(xi // CH)
            if xi % CH == 1 and xi // CH + 1 < nchunk:
                load_chunk(xi // CH + 1)
            xt = xt_at(xi)
            # policy logits: x part + h part (hT holds h_j)
            nc.tensor.matmul(l_ps[:], lhsT=xt, rhs=W[:, W_XP:W_XP + 16],
                             start=True, stop=False)
            nc.tensor.matmul(l_ps[:], lhsT=hT[:, 0:128],
                             rhs=W[:, W_HP:W_HP + 16], start=False, stop=False)
            nc.tensor.matmul(l_ps[:], lhsT=hT[:, 128:256],
                             rhs=W[:, W_HP + 16:W_HP + 32], start=False, stop=True)
            # gates: x2h early (accumulated in g_ps banks)
            nc.tensor.matmul(g_ps[:, 0:512], lhsT=xt, rhs=W[:, W_X2H:W_X2H + 512],
                             start=True, stop=False)
            nc.tensor.matmul(g_ps[:, 512:1024], lhsT=xt,
                             rhs=W[:, W_X2H + 512:W_X2H + 1024], start=True, stop=False)
            # sampling (chain-critical)
            nc.vector.scalar_tensor_tensor(LG[:], l_ps[:], 0.0, gum_at(j),
                                           AL.add, AL.add)
            nc.vector.max(mx[:], LG[:])
            nc.vector.max_index(mi[:], mx[:], LG[:])
            nc.vector.tensor_copy(act_f[:, j:j + 1], mi[:, 0:1])
            # ring-slot one-hot over 32 then fold mod 16
            cj = float((j + 1) % 16)
            nc.vector.tensor_scalar(m32[:], iota32, act_f[:, j:j + 1], cj,
                                    AL.subtract, AL.is_equal)
            nc.vector.tensor_tensor(m16[:], m32[:, 0:16], m32[:, 16:32], AL.add)
            # gather: mask-multiply-accumulate over live ring slots,
            # split between DVE and GPSIMD partial sums
            live = list(range(16)) if j >= 15 else list(range(j + 1))
            nd = ((len(live) * 5 + 4) // 8) | 1  # ~10/16 to DVE; odd so acc ends in A1
            dve_slots, gp_slots = live[:nd], live[nd:]
            r0s = dve_slots[0]
            nc.vector.tensor_scalar(A1[:], RING[:, r0s * 512:r0s * 512 + 512],
                                    m16[:, r0s:r0s + 1], None, AL.mult)
            cur = A1[:]
            other = S[:, 1024:1536]
            for r in dve_slots[1:]:
                nc.vector.scalar_tensor_tensor(
                    other, RING[:, r * 512:(r + 1) * 512], m16[:, r:r + 1], cur,
                    AL.mult, AL.add)
                cur, other = other, cur
            if gp_slots:
                rg = gp_slots[0]
                nc.gpsimd.tensor_tensor(G1[:], RING[:, rg * 512:rg * 512 + 512],
                                        m16[:, rg:rg + 1].to_broadcast([128, 512]),
                                        AL.mult)
                gcur, gother = G1[:], G2[:]
                for r in gp_slots[1:]:
                    nc.gpsimd.tensor_tensor(G3[:], RING[:, r * 512:(r + 1) * 512],
                                            m16[:, r:r + 1].to_broadcast([128, 512]),
                                            AL.mult)
                    nc.gpsimd.tensor_tensor(gother, gcur, G3[:], AL.add)
                    gcur, gother = gother, gcur
                nc.vector.tensor_tensor(S[:, 1024:1536], cur, gcur, AL.add)
            elif cur.tensor is not S.tensor:
                nc.vector.tensor_copy(S[:, 1024:1536], cur)
            # wc||wh in S[:, 1024:1536]
            transpose_256(S[:, 1280:1536], whT)
            # h2h accumulate into gates
            nc.tensor.matmul(g_ps[:, 0:512], lhsT=whT[:, 0:128],
                             rhs=W[:, W_H2H:W_H2H + 512], start=False, stop=False)
            nc.tensor.matmul(g_ps[:, 512:1024], lhsT=whT[:, 0:128],
                             rhs=W[:, W_H2H + 512:W_H2H + 1024], start=False, stop=False)
            nc.tensor.matmul(g_ps[:, 0:512], lhsT=whT[:, 128:256],
                             rhs=W[:, W_H2H + 1024:W_H2H + 1536], start=False, stop=True)
            nc.tensor.matmul(g_ps[:, 512:1024], lhsT=whT[:, 128:256],
                             rhs=W[:, W_H2H + 1536:W_H2H + 2048], start=False, stop=True)
            cell(j, with_h2h=True, g_ps=g_ps)
            # off-chain bookkeeping (after the chain ops so they fill gaps)
            nc.vector.tensor_copy(lstore[:, j * A:(j + 1) * A], l_ps[:])
            nc.vector.tensor_copy(act_i[:, j:j + 1], mi[:, 0:1])

            # ---- deferred logp for a finished chunk
            if (j + 1) % CH == 0 or j == nsteps - 1:
                c0 = (j // CH) * CH
                n = j + 1 - c0
                L = lstore[:, c0 * A:(j + 1) * A]
                L3 = L.rearrange("p (t a) -> p t a", a=A)
                nc.vector.tensor_reduce(r0[:, 0:n], L3, mybir.AxisListType.X, AL.max)
                nc.vector.tensor_tensor(
                    sc0[:, 0:n * A].rearrange("p (t a) -> p t a", a=A), L3,
                    r0[:, 0:n].unsqueeze(2).broadcast_to([128, n, A]), AL.subtract)
                nc.scalar.activation(sc1[:, 0:n * A], sc0[:, 0:n * A], AF.Exp)
                nc.vector.tensor_reduce(
                    r1[:, 0:n], sc1[:, 0:n * A].rearrange("p (t a) -> p t a", a=A),
                    mybir.AxisListType.X, AL.add)
                nc.scalar.activation(r1[:, 0:n], r1[:, 0:n], AF.Ln)
                # mask of chosen actions; sc0 still holds (L - rmax)
                nc.vector.tensor_tensor(
                    sc1[:, 0:n * A].rearrange("p (t a) -> p t a", a=A),
                    W[:, W_IOTA:W_IOTA + 16].unsqueeze(1).broadcast_to([128, n, A]),
                    act_f[:, c0:j + 1].unsqueeze(2).broadcast_to([128, n, A]),
                    AL.is_equal)
                nc.vector.tensor_tensor(sc0[:, 0:n * A], sc0[:, 0:n * A],
                                        sc1[:, 0:n * A], AL.mult)
                nc.vector.tensor_reduce(
                    r2[:, 0:n], sc0[:, 0:n * A].rearrange("p (t a) -> p t a", a=A),
                    mybir.AxisListType.X, AL.add)
                # logp = (sel - rmax...) wait: sel already = logit[a] - rmax
                nc.vector.tensor_tensor(probs[:, c0:j + 1], r2[:, 0:n], r1[:, 0:n],
                                        AL.subtract)

        # ---- epilogue: opt = softmax(h_last @ out_w.T); hT holds h_last^T
        nc.tensor.matmul(o_ps[:], lhsT=hT[:, 0:128], rhs=W[:, W_OUT:W_OUT + 128],
                         start=True, stop=False)
        nc.tensor.matmul(o_ps[:], lhsT=hT[:, 128:256], rhs=W[:, W_OUT + 128:W_OUT + 256],
                         start=False, stop=True)
        nc.vector.tensor_reduce(eps[:], o_ps[:], mybir.AxisListType.X, AL.max)
        nc.vector.tensor_scalar(eps[:], eps[:], -1.0, None, AL.mult)
        nc.scalar.activation(opt_t[:], o_ps[:], AF.Exp, bias=eps[:])
        nc.vector.tensor_reduce(eps[:], opt_t[:], mybir.AxisListType.X, AL.add)
        nc.vector.reciprocal(eps[:], eps[:])
        nc.vector.tensor_scalar(opt_t[:], opt_t[:], eps[:], None, AL.mult)

        nc.sync.dma_start(OUTF[:, 0:O], opt_t[:])
        nc.sync.dma_start(OUTF[:, O:O + nsteps], probs[:])
        nc.sync.dma_start(OUTI, act_i[:])
    return nc


# ---------------------------------------------------------------- host side

def _prep_weights(fc1_w, fc2_w, x2h_w, h2h_w, out_w):
    perm = np.concatenate([np.arange(0, 2 * H), np.arange(3 * H, 4 * H),
                           np.arange(2 * H, 3 * H)])  # [i,f,g,o] -> [i,f,o,g]
    wx2h = x2h_w[perm].T.astype(np.float32)            # [128, 1024]
    wh2h = h2h_w[perm].T.astype(np.float32)            # [256, 1024]
    wxp = (fc2_w @ fc1_w[:, :D]).T.astype(np.float32)  # [128, 16]
    whp = (fc2_w @ fc1_w[:, D:]).T.astype(np.float32)  # [256, 16]
    wout = out_w.T.astype(np.float32)                  # [256, 128]
    pack = np.zeros((128, W_END), np.float32)
    pack[:, W_X2H:W_X2H + 1024] = wx2h
    pack[:, W_H2H:W_H2H + 1024] = wh2h[0:128]
    pack[:, W_H2H + 1024:W_H2H + 2048] = wh2h[128:256]
    pack[:, W_XP:W_XP + 16] = wxp
    pack[:, W_HP:W_HP + 16] = whp[0:128]
    pack[:, W_HP + 16:W_HP + 32] = whp[128:256]
    pack[:, W_OUT:W_OUT + 128] = wout[0:128]
    pack[:, W_OUT + 128:W_OUT + 256] = wout[128:256]
    pack[:, W_IOTA:W_IOTA + 32] = np.arange(32, dtype=np.float32)[None, :]
    return pack


@functools.lru_cache(maxsize=1)
def _gumbel_all():
    import jax
    import jax.numpy as jnp
    with jax.default_device(jax.devices("cpu")[0]):
        keys = jax.random.split(jax.random.key(42), T)
        gs = [np.asarray(jax.random.gumbel(keys[jj], (B, A), jnp.float32))
              for jj in range(1, T)]
    return np.stack(gs)  # [255, B, 16]


_RUNNER = None


def _get_runner():
    global _RUNNER
    if _RUNNER is None:
        import jax
        from jax.sharding import Mesh, PartitionSpec
        from jax.experimental.shard_map import shard_map
        from concourse import bass2jax

        nc = build_bass()
        _split_sync_waits(nc)
        bass2jax.install_neuronx_cc_hook()
        partition_name = nc.partition_id_tensor.name if nc.partition_id_tensor else None
        in_names, out_names, out_avals, zero_outs = [], [], [], []
        for alloc in nc.m.functions[0].allocations:
            if not isinstance(alloc, mybir.MemoryLocationSet):
                continue
            name = alloc.memorylocations[0].name
            if alloc.kind == "ExternalInput":
                if name != partition_name:
                    in_names.append(name)
            elif alloc.kind == "ExternalOutput":
                out_names.append(name)
                shape = tuple(alloc.tensor_shape)
                dtype = mybir.dt.np(alloc.dtype)
                out_avals.append(jax.core.ShapedArray(shape, dtype))
                zero_outs.append(np.zeros(shape, dtype))
        n_params = len(in_names)
        n_outs = len(out_avals)
        in_names_all = in_names + out_names
        if partition_name is not None:
            in_names_all = in_names_all + [partition_name]
        donate = tuple(range(n_params, n_params + n_outs))

        def _body(*args):
            operands = list(args)
            if partition_name is not None:
                operands.append(bass2jax.partition_id_tensor())
            outs = bass2jax._bass_exec_p.bind(
                *operands, out_avals=tuple(out_avals), in_names=tuple(in_names_all),
                out_names=tuple(out_names), lowering_input_output_aliases=(),
                sim_require_finite=True, sim_require_nnan=True, nc=nc)
            return tuple(outs)

        devices = jax.devices()[:NCORES]
        mesh = Mesh(np.asarray(devices), ("core",))
        in_specs = (PartitionSpec("core"),) * (n_params + n_outs)
        out_specs = (PartitionSpec("core"),) * len(out_names)
        fn = jax.jit(shard_map(_body, mesh=mesh, in_specs=in_specs,
                               out_specs=out_specs, check_rep=False),
                     donate_argnums=donate, keep_unused=True)
        _RUNNER = (fn, in_names, out_names, out_avals, zero_outs)
    return _RUNNER


def _prep_ins(input, fc1_w, fc2_w, x2h_w, h2h_w, out_w):
    wpack = _prep_weights(np.asarray(fc1_w, np.float32), np.asarray(fc2_w, np.float32),
                          np.asarray(x2h_w, np.float32), np.asarray(h2h_w, np.float32),
                          np.asarray(out_w, np.float32))
    G = _gumbel_all()
    per_core = {"XT": [], "GUM": [], "WPK": []}
    for c in range(NCORES):
        rows = slice(c * BL, (c + 1) * BL)
        per_core["XT"].append(np.ascontiguousarray(input[rows].transpose(1, 2, 0)))
        per_core["GUM"].append(
            np.ascontiguousarray(G[:, rows, :].transpose(1, 0, 2).reshape(BL, -1)))
        per_core["WPK"].append(wpack)
    return per_core


def kernel_timed(input, fc1_w, fc1_b, fc2_w, fc2_b, x2h_w, x2h_b, h2h_w, h2h_b,
                 out_w, out_b, n_runs=5):
    """Run with device-resident inputs; returns (outputs, median_exec_seconds)."""
    import jax
    from jax.sharding import NamedSharding, PartitionSpec
    input = np.asarray(input, np.float32)
    per_core = _prep_ins(input, fc1_w, fc2_w, x2h_w, h2h_w, out_w)
    fn, in_names, out_names, out_avals, zero_outs = _get_runner()
    ins_np = [np.concatenate(per_core[name], axis=0) for name in in_names]
    mesh = fn._mesh if hasattr(fn, "_mesh") else None
    from jax.sharding import Mesh
    devices = jax.devices()[:NCORES]
    mesh = Mesh(np.asarray(devices), ("core",))
    sh = NamedSharding(mesh, PartitionSpec("core"))
    ins_dev = [jax.device_put(a, sh) for a in ins_np]
    jax.block_until_ready(ins_dev)

    def once():
        zeros = [np.zeros((NCORES * z.shape[0], *z.shape[1:]), z.dtype)
                 for z in zero_outs]
        t0 = time.time()
        out_arrs = fn(*ins_dev, *zeros)
        jax.block_until_ready(out_arrs)
        return time.time() - t0, out_arrs

    once()  # warm
    times = []
    out_arrs = None
    for _ in range(n_runs):
        dt, out_arrs = once()
        times.append(dt)
    med = sorted(times)[len(times) // 2]
    outs = {name: np.asarray(out_arrs[i]) for i, name in enumerate(out_names)}
    outf = outs["OUTF"].reshape(NCORES, BL, O + NSTEP)
    outi = outs["OUTI"].reshape(NCORES, BL, NSTEP)
    opt = outf[:, :, 0:O].reshape(B, O)
    probs = outf[:, :, O:].reshape(B, NSTEP).T.copy()
    actions = outi.reshape(B, NSTEP).T.astype(np.int32).copy()
    return (opt, actions, probs), med, times


def kernel(input, fc1_w, fc1_b, fc2_w, fc2_b, x2h_w, x2h_b, h2h_w, h2h_b,
           out_w, out_b):
    input = np.asarray(input, np.float32)
    for bias in (fc1_b, fc2_b, x2h_b, h2h_b, out_b):
        assert np.all(np.asarray(bias) == 0.0), "kernel assumes zero biases"
    wpack = _prep_weights(np.asarray(fc1_w, np.float32), np.asarray(fc2_w, np.float32),
                          np.asarray(x2h_w, np.float32), np.asarray(h2h_w, np.float32),
                          np.asarray(out_w, np.float32))
    G = _gumbel_all()  # [255, B, 16]

    fn, in_names, out_names, out_avals, zero_outs = _get_runner()

    per_core = {"XT": [], "GUM": [], "WPK": []}
    for c in range(NCORES):
        rows = slice(c * BL, (c + 1) * BL)
        per_core["XT"].append(np.ascontiguousarray(input[rows].transpose(1, 2, 0)))
        per_core["GUM"].append(
            np.ascontiguousarray(G[:, rows, :].transpose(1, 0, 2).reshape(BL, -1)))
        per_core["WPK"].append(wpack)
    ins = [np.concatenate(per_core[name], axis=0) for name in in_names]
    zeros = [np.zeros((NCORES * z.shape[0], *z.shape[1:]), z.dtype) for z in zero_outs]
    out_arrs = fn(*ins, *zeros)
    import jax
    jax.block_until_ready(out_arrs)
    outs = {name: np.asarray(out_arrs[i]) for i, name in enumerate(out_names)}

    outf = outs["OUTF"].reshape(NCORES, BL, O + NSTEP)
    outi = outs["OUTI"].reshape(NCORES, BL, NSTEP)
    opt = outf[:, :, 0:O].reshape(B, O)
    probs = outf[:, :, O:].reshape(B, NSTEP).T.copy()
    actions = outi.reshape(B, NSTEP).T.astype(np.int32).copy()
    return opt, actions, probs
